# revision 19
# baseline (speedup 1.0000x reference)
"""Bidirectional Mamba layer on 8 Trainium2 NeuronCores (Bass/Tile).

Sharding: 8 cores = 2 directions x 4 batch samples. Each core runs the full
Mamba block for its (direction, sample) pair; a pairwise AllGather combines
the two directions, and every core (redundantly, SPMD-symmetric) applies the
final residual + LayerNorm.

Layout convention on device: channels on partitions, time on the free axis.
The selective scan runs as DVE tensor_tensor_scan (one recurrence per
(channel, state) partition row, time along free). Per channel-block g the
16 state recurrences share batched B/C multiplies: bt_all/yn_all are single
[128, DS*TC] DVE passes using a stride-0 free broadcast of p / direct reads
of a per-chunk broadcast tile of B and C rows.
"""

import ml_dtypes
import numpy as np

import concourse.bass as bass
import concourse.bacc as bacc
import concourse.tile as tile
from concourse import mybir
from concourse.bass_utils import run_bass_kernel_spmd

# ---- problem shapes (hardcoded per contract) ----
B_SZ, L, DM = 4, 2048, 512
D_CONV, DS, DR = 4, 16, 32
DI = 1024                  # d_inner
E2 = 2 * DI                # in_proj rows
NG = DI // 128             # 8 channel blocks
TC = 512                   # time chunk
NCH = L // TC              # 4 chunks
NTT = L // 128             # 16 time tiles of 128
EPS = 1e-5
F32 = mybir.dt.float32
F32R = mybir.dt.float32r
BF16 = mybir.dt.bfloat16
AF = mybir.ActivationFunctionType
OP = mybir.AluOpType

N_CORES = 8

# CoreSim does not implement Silu; tests flip this to use sigmoid+mult
USE_SILU = True
REPLICA_GROUPS = [[0, 4], [1, 5], [2, 6], [3, 7]]


def _silu(nc, pool, out_tile, psum, bias):
    """out = silu(psum + bias); Silu on HW, sigmoid+mult fallback for CoreSim."""
    if USE_SILU:
        nc.scalar.activation(out_tile, psum, AF.Silu, bias=bias)
    else:
        zb = pool.tile(list(out_tile.shape), F32, tag="_silu_zb", name="zb",
                       bufs=1)
        nc.scalar.activation(zb, psum, AF.Identity, bias=bias)
        nc.scalar.activation(out_tile, psum, AF.Sigmoid, bias=bias)
        nc.vector.tensor_mul(out_tile, zb, out_tile)


def build_module():
    nc = bacc.Bacc(
        "TRN2", target_bir_lowering=False, debug=False, num_devices=N_CORES
    )

    # ---------------- I/O ----------------
    x_d = nc.dram_tensor("x_d", [L, DM], BF16, kind="ExternalInput")
    x_nat = nc.dram_tensor("x_nat", [L, DM], F32, kind="ExternalInput")
    w_in = nc.dram_tensor("w_in", [DM, E2], BF16, kind="ExternalInput")
    convd = nc.dram_tensor("convd", [D_CONV, NG, 128, 128], BF16, kind="ExternalInput")
    convb = nc.dram_tensor("convb", [NG, 128], F32, kind="ExternalInput")
    silub = nc.dram_tensor("silub", [NG, 128], F32, kind="ExternalInput")
    w_xp = nc.dram_tensor("w_xp", [DI, DR + 2 * DS], BF16, kind="ExternalInput")
    w_dt = nc.dram_tensor("w_dt", [DR, DI], F32R, kind="ExternalInput")
    dtb = nc.dram_tensor("dtb", [NG, 128], F32, kind="ExternalInput")
    a_sc = nc.dram_tensor("a_sc", [NG, 128, DS], F32, kind="ExternalInput")
    d_diag = nc.dram_tensor("d_diag", [NG, 128, 128], BF16, kind="ExternalInput")
    w_out = nc.dram_tensor("w_out", [DI, DM], BF16, kind="ExternalInput")
    eye = nc.dram_tensor("eye", [128, 128], BF16, kind="ExternalInput")
    eyen = nc.dram_tensor("eyen", [128, 128], BF16, kind="ExternalInput")
    eyej = nc.dram_tensor("eyej", [128, 128], BF16, kind="ExternalInput")
    ln2w = nc.dram_tensor("ln2w", [1, DM], F32, kind="ExternalInput")
    ln2b = nc.dram_tensor("ln2b", [1, DM], F32, kind="ExternalInput")
    out = nc.dram_tensor("out", [L, DM], F32, kind="ExternalOutput")

    with tile.TileContext(nc) as tc:
        build_program(
            tc, x_d, x_nat, w_in, convd, convb, silub, w_xp, w_dt, dtb,
            a_sc, d_diag, w_out, eye, eyen, eyej, ln2w, ln2b, out,
        )
    nc.compile()
    return nc


def build_program(tc, x_d, x_nat, w_in, convd, convb, silub, w_xp, w_dt, dtb,
                  a_sc, d_diag, w_out, eye, eyen, eyej, ln2w, ln2b, out):
    nc = tc.nc
    from contextlib import ExitStack

    ctx = ExitStack()
    with ctx:
        wp = ctx.enter_context(tc.tile_pool(name="wp", bufs=1))
        lnp = ctx.enter_context(tc.tile_pool(name="lnp", bufs=2))
        chp = ctx.enter_context(tc.tile_pool(name="chp", bufs=1))
        snp = ctx.enter_context(tc.tile_pool(name="snp", bufs=2))
        pp = ctx.enter_context(tc.tile_pool(name="pp", space="PSUM", bufs=2))
        ppy = ctx.enter_context(tc.tile_pool(name="ppy", space="PSUM", bufs=2))
        dramp = ctx.enter_context(tc.tile_pool(name="dramp", bufs=2, space="DRAM"))

        # ---------------- prefetch chunk-0 input ahead of the weights ----
        xt4_pre = chp.tile([128, TC // 128, DM], BF16, tag="xt4", bufs=1)
        for i in range(TC // 128):
            nc.sync.dma_start(xt4_pre[:, i, :],
                              x_d[i * 128:(i + 1) * 128, :])

        # ---------------- load weights ----------------
        w_in_sb = wp.tile([128, DM // 128, E2], BF16)
        nc.sync.dma_start(w_in_sb, w_in.rearrange("(k p) e -> p k e", p=128))
        convd_sb = wp.tile([128, D_CONV, NG, 128], BF16)
        for k in range(D_CONV):
            nc.sync.dma_start(convd_sb[:, k, :, :], convd[k].rearrange("g p e -> p g e"))
        w_xp_sb = wp.tile([128, NG, DR + 2 * DS], BF16)
        nc.sync.dma_start(w_xp_sb, w_xp.rearrange("(g p) e -> p g e", p=128))
        w_dt_sb = wp.tile([DR, DI], F32R)
        nc.sync.dma_start(w_dt_sb, w_dt[:, :])
        w_out_sb = wp.tile([128, NG, DM], BF16)
        nc.sync.dma_start(w_out_sb, w_out.rearrange("(g p) e -> p g e", p=128))
        a_sb = wp.tile([128, NG, DS], F32)
        nc.sync.dma_start(a_sb, a_sc.rearrange("g p e -> p g e"))
        d_diag_sb = wp.tile([128, NG, 128], BF16)
        nc.sync.dma_start(d_diag_sb, d_diag.rearrange("g p e -> p g e"))
        eye_sb = wp.tile([128, 128], BF16)
        nc.sync.dma_start(eye_sb, eye[:, :])
        eyen_sb = wp.tile([128, 128], BF16)
        nc.sync.dma_start(eyen_sb, eyen[:, :])
        eyej_sb = wp.tile([128, 128], BF16)
        nc.sync.dma_start(eyej_sb, eyej[:, :])

        def col_load(name, src):
            t = wp.tile([128, NG], F32, name=name)
            for g in range(NG):
                nc.sync.dma_start(t[:, g:g + 1], src[g:g + 1, :].rearrange("a b -> b a"))
            return t

        convb_sb = col_load("convb_sb", convb)
        silub_sb = col_load("silub_sb", silub)
        dtb_sb = col_load("dtb_sb", dtb)

        ln2w_sb = wp.tile([128, DM], F32)
        nc.sync.dma_start(ln2w_sb, ln2w[0:1, :].to_broadcast([128, DM]))
        ln2b_sb = wp.tile([128, DM], F32)
        nc.sync.dma_start(ln2b_sb, ln2b[0:1, :].to_broadcast([128, DM]))

        eps_sb = wp.tile([128, 1], F32)
        nc.vector.memset(eps_sb, EPS)
        # per-(g,n) scan carry, column n of block g; zero before chunk 0
        hlast = wp.tile([128, NG, DS], BF16)
        nc.vector.memset(hlast, 0.0)

        ypart = dramp.tile([L, DM], BF16, bufs=1)
        gbuf = dramp.tile([NCH, 2, TC, DM], BF16, bufs=1)

        # conv staging with 3-column causal halo; persistent across chunks so
        # the halo copy reads the previous chunk's columns from the same tile
        xz = [chp.tile([128, TC + 3], BF16, name=f"xz{g}") for g in range(NG)]

        # ---------------- per-chunk pipeline ----------------
        for c in range(NCH):
            t0 = c * TC

            # -- A: LayerNorm 1 (affine folded into weights) + transpose --
            # activations batched across the 4 time-tiles (all Ln, then all
            # Exp) to avoid per-tile ACT table-set reloads.
            xn_t = chp.tile([128, DM // 128, TC], BF16, tag="xn_t")
            if c == 0:
                xt4 = xt4_pre
            else:
                xt4 = chp.tile([128, TC // 128, DM], BF16, tag="xt4", bufs=1)
                for i in range(TC // 128):
                    nc.sync.dma_start(xt4[:, i, :],
                                      x_d[t0 + i * 128: t0 + (i + 1) * 128, :])
            mv4 = lnp.tile([128, TC // 128, 2], F32, tag="mv4")
            for i in range(TC // 128):
                st = lnp.tile([128, 6], F32, tag="st")
                nc.vector.bn_stats(st, xt4[:, i, :])
                nc.vector.bn_aggr(mv4[:, i, :], st)
            for i in range(TC // 128):
                nc.scalar.activation(mv4[:, i, 1:2], mv4[:, i, 1:2], AF.Ln,
                                     bias=eps_sb[:, 0:1])
            for i in range(TC // 128):
                nc.scalar.activation(mv4[:, i, 1:2], mv4[:, i, 1:2], AF.Exp,
                                     scale=-0.5)
            for i in range(TC // 128):
                nc.vector.tensor_scalar(out=xt4[:, i, :], in0=xt4[:, i, :],
                                        scalar1=mv4[:, i, 0:1],
                                        scalar2=mv4[:, i, 1:2],
                                        op0=OP.subtract, op1=OP.mult)
                ps_t = pp.tile([128, DM], BF16, tag="trps", bufs=1)
                for kb in range(DM // 128):
                    nc.tensor.transpose(ps_t[:, kb * 128:(kb + 1) * 128],
                                        xt4[:, i, kb * 128:(kb + 1) * 128],
                                        eye_sb)
                nc.vector.tensor_copy(
                    xn_t[:, :, i * 128:(i + 1) * 128],
                    ps_t.rearrange("p (k t) -> p k t", k=DM // 128))

            # -- B: in_proj (xc half) into conv staging with halo --
            for g in range(NG):
                if c == 0:
                    nc.vector.memset(xz[g][:, 0:3], 0.0)
                else:
                    nc.vector.tensor_copy(xz[g][:, 0:3], xz[g][:, TC:TC + 3])
                pz = pp.tile([128, TC], F32, tag="ps")
                for k in range(DM // 128):
                    nc.tensor.matmul(pz, w_in_sb[:, k, g * 128:(g + 1) * 128],
                                     xn_t[:, k, :],
                                     start=(k == 0), stop=(k == DM // 128 - 1))
                nc.scalar.copy(xz[g][:, 3:TC + 3], pz)

            # -- B2: in_proj (z half) + SiLU gate --
            sg = [chp.tile([128, TC], BF16, tag=f"sg{g}", name=f"sg{g}") for g in range(NG)]
            for g in range(NG):
                pz = pp.tile([128, TC], F32, tag="ps")
                for k in range(DM // 128):
                    nc.tensor.matmul(
                        pz, w_in_sb[:, k, DI + g * 128: DI + (g + 1) * 128],
                        xn_t[:, k, :],
                        start=(k == 0), stop=(k == DM // 128 - 1))
                _silu(nc, snp, sg[g], pz, silub_sb[:, g:g + 1])

            # -- C: depthwise causal conv + SiLU --
            xc = [chp.tile([128, TC], BF16, tag=f"xc{g}", name=f"xc{g}") for g in range(NG)]
            for g in range(NG):
                pc = pp.tile([128, TC], F32, tag="ps")
                for k in range(D_CONV):
                    nc.tensor.matmul(pc, convd_sb[:, k, g, :],
                                     xz[g][:, k:k + TC],
                                     start=(k == 0), stop=(k == D_CONV - 1))
                _silu(nc, snp, xc[g], pc, convb_sb[:, g:g + 1])

            # -- D: x_proj -> (dt_r, B, C) --
            pxd = pp.tile([64, TC], F32, tag="pxd", bufs=1)
            for g in range(NG):
                nc.tensor.matmul(pxd, w_xp_sb[:, g, :], xc[g],
                                 start=(g == 0), stop=(g == NG - 1))
            xdbl = chp.tile([64, TC], F32R, tag="xdbl", bufs=2)
            nc.scalar.copy(xdbl, pxd)
            bc_d = dramp.tile([2 * DS, TC], BF16, tag="bc_d", bufs=2)
            bc_bf = chp.tile([2 * DS, TC], BF16, tag="bc_bf", bufs=2)
            nc.scalar.copy(bc_bf, xdbl[DR:DR + 2 * DS, :])
            nc.sync.dma_start(bc_d, bc_bf)
            # hoisted per-chunk broadcast of all B (rows 0:DS) and C rows
            # (DS:2*DS) across the 128 partitions
            bcrep = chp.tile([128, 2 * DS, TC], BF16, tag="bcrep", bufs=1)
            nc.sync.dma_start(
                bcrep,
                bc_d.rearrange("n t -> (n t)")[None, :].to_broadcast(
                    [128, 2 * DS * TC]).rearrange("p (n t) -> p n t", n=2 * DS))

            # -- D2: dt = softplus(dt_proj @ dt_r + bias); P = dt * xc --
            # batched by ACT table-set: all Exp first, then all Ln, so the
            # activation table is not reloaded per group.
            dt_c = [chp.tile([128, TC], BF16, tag=f"dt{g}", name=f"dtc{g}") for g in range(NG)]
            p_c = [chp.tile([128, TC], BF16, tag=f"p{g}", name=f"pc{g}") for g in range(NG)]
            for g in range(NG):
                pdt = pp.tile([128, TC], F32, tag="ps")
                nc.tensor.matmul(pdt, w_dt_sb[:, g * 128:(g + 1) * 128],
                                 xdbl[0:DR, :], start=True, stop=True)
                nc.scalar.activation(dt_c[g], pdt, AF.Exp,
                                     bias=dtb_sb[:, g:g + 1])
            for g in range(NG):
                nc.scalar.activation(dt_c[g], dt_c[g], AF.Ln, bias=1.0)
            for g in range(NG):
                nc.vector.tensor_mul(p_c[g], dt_c[g], xc[g])

            # -- E/F: selective scan, one channel block at a time --
            yf = [chp.tile([128, TC], BF16, tag=f"yf{g}", name=f"yf{g}") for g in range(NG)]
            prev_gate = None
            for g in range(NG):
                bt_all = chp.tile([128, DS, TC], BF16, tag="bt_all",
                                  name=f"bt{g}", bufs=1)
                nc.vector.tensor_mul(
                    bt_all,
                    p_c[g][:, None, :].to_broadcast([128, DS, TC]),
                    bcrep[:, 0:DS, :])
                h_all = chp.tile([128, DS, TC], BF16, tag="h_all",
                                 name=f"h{g}", bufs=1)
                py = ppy.tile([128, TC], F32, tag="py", bufs=2)
                # seed with the D-term via a diagonal matmul: py = diag(D) @ xc
                nc.tensor.matmul(py, d_diag_sb[:, g, :], xc[g],
                                 start=True, stop=False)
                for n in range(DS):
                    da = snp.tile([128, TC], F32, tag="da")
                    nc.scalar.activation(da, dt_c[g], AF.Exp,
                                         scale=a_sb[:, g, n:n + 1])
                    nc.vector.tensor_tensor_scan(
                        h_all[:, n, :], da, bt_all[:, n, :],
                        initial=hlast[:, g, n:n + 1],
                        op0=OP.mult, op1=OP.add)
                # save the per-state carry with an on-queue DVE copy: a DMA
                # here costs ~5us of cross-engine semaphore latency at every
                # block boundary (the next block's scans WAR on h_all)
                nc.vector.tensor_copy(hlast[:, g, :], h_all[:, :, TC - 1])
                yn_all = chp.tile([128, DS, TC], BF16, tag="yn_all",
                                  name=f"yn{g}", bufs=1)
                nc.vector.tensor_mul(yn_all, h_all, bcrep[:, DS:2 * DS, :])
                for n in range(DS):
                    nc.tensor.matmul(py, eyen_sb, yn_all[:, n, :],
                                     start=False, stop=(n == DS - 1))
                # gate the PREVIOUS block now: its y-acc matmuls ran while this
                # block scanned, so the DVE never stalls on the tensor engine
                if prev_gate is not None:
                    pg, ppy_t = prev_gate
                    nc.vector.tensor_mul(yf[pg], ppy_t, sg[pg])
                prev_gate = (g, py)
            pg, ppy_t = prev_gate
            nc.vector.tensor_mul(yf[pg], ppy_t, sg[pg])

            # -- H: out_proj, emitted directly in [t, dm] layout --
            for tb in range(TC // 128):
                po = pp.tile([128, DM], F32, tag="ps")
                for g in range(NG):
                    nc.tensor.matmul(po, yf[g][:, tb * 128:(tb + 1) * 128],
                                     w_out_sb[:, g, :],
                                     start=(g == 0), stop=(g == NG - 1))
                ot = lnp.tile([128, DM], BF16, tag="ot2")
                nc.scalar.copy(ot, po)
                nc.sync.dma_start(
                    ypart[t0 + tb * 128: t0 + (tb + 1) * 128, :], ot)
            # per-chunk AllGather of this chunk's partial output; overlaps
            # with the next chunk's compute, so only the last one is a tail
            nc.gpsimd.collective_compute(
                "AllGather", OP.bypass, replica_groups=REPLICA_GROUPS,
                ins=[ypart[t0:t0 + TC, :].opt()], outs=[gbuf[c].opt()],
            )

        # ---------------- combine directions + final LayerNorm ----------------
        # Natural chunk cn reads the dir-0 partial from gbuf[cn, 0] and the
        # dir-1 (flipped) partial from gbuf[NCH-1-cn, 1]. Chunks 1 and 2 need
        # only the earlier collectives, so process them first; only chunks
        # {3, 0} wait on the last collective.
        for cn in (1, 2, 3, 0):
            for it in range(TC // 128):
                i = cn * (TC // 128) + it
                c1 = NCH - 1 - cn
                o1 = TC - (it + 1) * 128
                s0 = lnp.tile([128, DM], BF16, tag="xt")
                nc.sync.dma_start(s0, gbuf[cn, 0, it * 128:(it + 1) * 128, :])
                # dir-1 partial is in flipped time order: load the mirrored
                # block forward, then reverse rows via the exchange matrix on
                # the tensor engine (DMA cannot step partitions backwards)
                s1 = lnp.tile([128, DM], BF16, tag="ot")
                nc.sync.dma_start(s1, gbuf[c1, 1, o1:o1 + 128, :])
                pj = pp.tile([128, DM], F32, tag="ps")
                nc.tensor.matmul(pj, eyej_sb, s1, start=True, stop=True)
                xt2 = lnp.tile([128, DM], F32, tag="xt2")
                nc.sync.dma_start(xt2, x_nat[i * 128:(i + 1) * 128, :])
                sf = lnp.tile([128, DM], F32, tag="sf")
                nc.vector.tensor_add(sf, pj, s0)
                nc.vector.tensor_add(sf, sf, xt2)
                st = lnp.tile([128, 6], F32, tag="st")
                nc.vector.bn_stats(st, sf)
                mv = lnp.tile([128, 2], F32, tag="mv")
                nc.vector.bn_aggr(mv, st)
                nc.scalar.activation(mv[:, 1:2], mv[:, 1:2], AF.Ln,
                                     bias=eps_sb[:, 0:1])
                nc.scalar.activation(mv[:, 1:2], mv[:, 1:2], AF.Exp, scale=-0.5)
                nc.vector.tensor_scalar(out=sf, in0=sf, scalar1=mv[:, 0:1],
                                        scalar2=mv[:, 1:2],
                                        op0=OP.subtract, op1=OP.mult)
                nc.vector.tensor_mul(sf, sf, ln2w_sb)
                nc.vector.tensor_add(sf, sf, ln2b_sb)
                nc.sync.dma_start(out[i * 128:(i + 1) * 128, :], sf)


# ---------------- host side ----------------

def make_core_inputs(inputs):
    """Build the 8 per-core input dicts from the full problem inputs."""
    x = np.ascontiguousarray(inputs["x"], dtype=np.float32)        # [B, L, DM]
    ln1_w = inputs["ln1_w"].astype(np.float32)
    ln1_b = inputs["ln1_b"].astype(np.float32)
    in_w = inputs["in_proj_w"].astype(np.float32)                  # [2, 2DI, DM]
    conv_w = inputs["conv_w"].astype(np.float32)                   # [2, DI, 4]
    conv_b = inputs["conv_b"].astype(np.float32)                   # [2, DI]
    xp_w = inputs["x_proj_w"].astype(np.float32)                   # [2, 64, DI]
    dt_w = inputs["dt_proj_w"].astype(np.float32)                  # [2, DI, DR]
    dt_b = inputs["dt_proj_b"].astype(np.float32)                  # [2, DI]
    a_log = inputs["A_log"].astype(np.float32)                     # [2, DI, DS]
    d_par = inputs["D_param"].astype(np.float32)                   # [2, DI]
    out_w = inputs["out_proj_w"].astype(np.float32)                # [2, DM, DI]
    ln2_w = inputs["ln2_w"].astype(np.float32)
    ln2_b = inputs["ln2_b"].astype(np.float32)

    bf16 = ml_dtypes.bfloat16
    eye = np.eye(128, dtype=np.float32)
    per_dir = []
    for d in range(2):
        w = in_w[d]                                   # [2DI, DM]
        w_in_T = np.ascontiguousarray((w * ln1_w[None, :]).T)      # [DM, 2DI]
        v = w @ ln1_b                                  # [2DI]
        csum = conv_w[d].sum(axis=1)                   # [DI]
        convb_adj = conv_b[d] + csum * v[:DI]
        silub_adj = v[DI:]
        convd = np.zeros((D_CONV, NG, 128, 128), np.float32)
        for k in range(D_CONV):
            for g in range(NG):
                np.fill_diagonal(convd[k, g], conv_w[d, g * 128:(g + 1) * 128, k])
        d_diag = np.zeros((NG, 128, 128), np.float32)
        for g in range(NG):
            np.fill_diagonal(d_diag[g], d_par[d, g * 128:(g + 1) * 128])
        a_neg = -np.exp(a_log[d])                      # [DI, DS]
        per_dir.append(dict(
            w_in=w_in_T.astype(bf16),
            convd=convd.astype(bf16),
            convb=convb_adj.reshape(NG, 128),
            silub=silub_adj.reshape(NG, 128),
            w_xp=np.ascontiguousarray(xp_w[d].T).astype(bf16),     # [DI, 64]
            w_dt=np.ascontiguousarray(dt_w[d].T),      # [DR, DI]
            dtb=dt_b[d].reshape(NG, 128),
            a_sc=np.ascontiguousarray(a_neg.reshape(NG, 128, DS)),
            d_diag=d_diag.astype(bf16),
            w_out=np.ascontiguousarray(out_w[d].T).astype(bf16),   # [DI, DM]
        ))

    in_maps = []
    for core in range(N_CORES):
        d, b = core // 4, core % 4
        xb = x[b]
        m = dict(per_dir[d])
        m["x_d"] = (xb if d == 0 else np.ascontiguousarray(xb[::-1])).astype(bf16)
        m["x_nat"] = xb
        m["eye"] = eye.astype(bf16)
        m["eyen"] = eye.astype(bf16)
        m["eyej"] = eye[::-1].astype(bf16)
        m["ln2w"] = ln2_w.reshape(1, DM)
        m["ln2b"] = ln2_b.reshape(1, DM)
        in_maps.append(m)
    return in_maps


_NC = None


def _get_module():
    global _NC
    if _NC is None:
        _NC = build_module()
    return _NC


def kernel(**inputs) -> np.ndarray:
    nc = _get_module()
    in_maps = make_core_inputs(inputs)
    res = run_bass_kernel_spmd(nc, in_maps, core_ids=list(range(N_CORES)))
    outs = [res.results[b]["out"] for b in range(B_SZ)]
    return np.stack(outs, axis=0)


if __name__ == "__main__":
    nc = build_module()
    print("module built ok")


# revision 20
# speedup vs baseline: 1.0192x; 1.0192x over previous
"""Bidirectional Mamba layer on 8 Trainium2 NeuronCores (Bass/Tile).

Sharding: 8 cores = 2 directions x 4 batch samples. Each core runs the full
Mamba block for its (direction, sample) pair; a pairwise AllGather combines
the two directions, and every core (redundantly, SPMD-symmetric) applies the
final residual + LayerNorm.

Layout convention on device: channels on partitions, time on the free axis.
The selective scan runs as DVE tensor_tensor_scan (one recurrence per
(channel, state) partition row, time along free). Per channel-block g the
16 state recurrences share batched B/C multiplies: bt_all/yn_all are single
[128, DS*TC] DVE passes using a stride-0 free broadcast of p / direct reads
of a per-chunk broadcast tile of B and C rows.
"""

import ml_dtypes
import numpy as np

import concourse.bass as bass
import concourse.bacc as bacc
import concourse.tile as tile
from concourse import mybir
from concourse.bass_utils import run_bass_kernel_spmd

# ---- problem shapes (hardcoded per contract) ----
B_SZ, L, DM = 4, 2048, 512
D_CONV, DS, DR = 4, 16, 32
DI = 1024                  # d_inner
E2 = 2 * DI                # in_proj rows
NG = DI // 128             # 8 channel blocks
TC = 512                   # time chunk
NCH = L // TC              # 4 chunks
NTT = L // 128             # 16 time tiles of 128
EPS = 1e-5
F32 = mybir.dt.float32
F32R = mybir.dt.float32r
BF16 = mybir.dt.bfloat16
AF = mybir.ActivationFunctionType
OP = mybir.AluOpType

N_CORES = 8

# CoreSim does not implement Silu; tests flip this to use sigmoid+mult
USE_SILU = True
REPLICA_GROUPS = [[0, 4], [1, 5], [2, 6], [3, 7]]


def _silu(nc, pool, out_tile, psum, bias):
    """out = silu(psum + bias); Silu on HW, sigmoid+mult fallback for CoreSim."""
    if USE_SILU:
        nc.scalar.activation(out_tile, psum, AF.Silu, bias=bias)
    else:
        zb = pool.tile(list(out_tile.shape), F32, tag="_silu_zb", name="zb",
                       bufs=1)
        nc.scalar.activation(zb, psum, AF.Identity, bias=bias)
        nc.scalar.activation(out_tile, psum, AF.Sigmoid, bias=bias)
        nc.vector.tensor_mul(out_tile, zb, out_tile)


def build_module():
    nc = bacc.Bacc(
        "TRN2", target_bir_lowering=False, debug=False, num_devices=N_CORES
    )

    # ---------------- I/O ----------------
    x_d = nc.dram_tensor("x_d", [L, DM], BF16, kind="ExternalInput")
    x_nat = nc.dram_tensor("x_nat", [L, DM], F32, kind="ExternalInput")
    w_in = nc.dram_tensor("w_in", [DM, E2], BF16, kind="ExternalInput")
    convd = nc.dram_tensor("convd", [D_CONV, NG, 128, 128], BF16, kind="ExternalInput")
    convb = nc.dram_tensor("convb", [NG, 128], F32, kind="ExternalInput")
    silub = nc.dram_tensor("silub", [NG, 128], F32, kind="ExternalInput")
    w_xp = nc.dram_tensor("w_xp", [DI, DR + 2 * DS], BF16, kind="ExternalInput")
    w_dt = nc.dram_tensor("w_dt", [DR, DI], F32R, kind="ExternalInput")
    dtb = nc.dram_tensor("dtb", [NG, 128], F32, kind="ExternalInput")
    a_sc = nc.dram_tensor("a_sc", [NG, 128, DS], F32, kind="ExternalInput")
    d_diag = nc.dram_tensor("d_diag", [NG, 128, 128], BF16, kind="ExternalInput")
    w_out = nc.dram_tensor("w_out", [DI, DM], BF16, kind="ExternalInput")
    eye = nc.dram_tensor("eye", [128, 128], BF16, kind="ExternalInput")
    eyen = nc.dram_tensor("eyen", [128, 128], BF16, kind="ExternalInput")
    eyej = nc.dram_tensor("eyej", [128, 128], BF16, kind="ExternalInput")
    ln2w = nc.dram_tensor("ln2w", [1, DM], F32, kind="ExternalInput")
    ln2b = nc.dram_tensor("ln2b", [1, DM], F32, kind="ExternalInput")
    out = nc.dram_tensor("out", [L, DM], F32, kind="ExternalOutput")

    with tile.TileContext(nc) as tc:
        build_program(
            tc, x_d, x_nat, w_in, convd, convb, silub, w_xp, w_dt, dtb,
            a_sc, d_diag, w_out, eye, eyen, eyej, ln2w, ln2b, out,
        )
    nc.compile()
    return nc


def build_program(tc, x_d, x_nat, w_in, convd, convb, silub, w_xp, w_dt, dtb,
                  a_sc, d_diag, w_out, eye, eyen, eyej, ln2w, ln2b, out):
    nc = tc.nc
    from contextlib import ExitStack

    ctx = ExitStack()
    with ctx:
        wp = ctx.enter_context(tc.tile_pool(name="wp", bufs=1))
        lnp = ctx.enter_context(tc.tile_pool(name="lnp", bufs=2))
        chp = ctx.enter_context(tc.tile_pool(name="chp", bufs=1))
        snp = ctx.enter_context(tc.tile_pool(name="snp", bufs=2))
        pp = ctx.enter_context(tc.tile_pool(name="pp", space="PSUM", bufs=2))
        ppy = ctx.enter_context(tc.tile_pool(name="ppy", space="PSUM", bufs=2))
        dramp = ctx.enter_context(tc.tile_pool(name="dramp", bufs=2, space="DRAM"))

        # ---------------- prefetch chunk-0 input ahead of the weights ----
        xt4_pre = chp.tile([128, TC // 128, DM], BF16, tag="xt4", bufs=1)
        for i in range(TC // 128):
            nc.sync.dma_start(xt4_pre[:, i, :],
                              x_d[i * 128:(i + 1) * 128, :])

        # ---------------- load weights ----------------
        w_in_sb = wp.tile([128, DM // 128, E2], BF16)
        nc.sync.dma_start(w_in_sb, w_in.rearrange("(k p) e -> p k e", p=128))
        convd_sb = wp.tile([128, D_CONV, NG, 128], BF16)
        for k in range(D_CONV):
            nc.sync.dma_start(convd_sb[:, k, :, :], convd[k].rearrange("g p e -> p g e"))
        w_xp_sb = wp.tile([128, NG, DR + 2 * DS], BF16)
        nc.sync.dma_start(w_xp_sb, w_xp.rearrange("(g p) e -> p g e", p=128))
        w_dt_sb = wp.tile([DR, DI], F32R)
        nc.sync.dma_start(w_dt_sb, w_dt[:, :])
        w_out_sb = wp.tile([128, NG, DM], BF16)
        nc.sync.dma_start(w_out_sb, w_out.rearrange("(g p) e -> p g e", p=128))
        a_sb = wp.tile([128, NG, DS], F32)
        nc.sync.dma_start(a_sb, a_sc.rearrange("g p e -> p g e"))
        d_diag_sb = wp.tile([128, NG, 128], BF16)
        nc.sync.dma_start(d_diag_sb, d_diag.rearrange("g p e -> p g e"))
        eye_sb = wp.tile([128, 128], BF16)
        nc.sync.dma_start(eye_sb, eye[:, :])
        eyen_sb = wp.tile([128, 128], BF16)
        nc.sync.dma_start(eyen_sb, eyen[:, :])
        eyej_sb = wp.tile([128, 128], BF16)
        nc.sync.dma_start(eyej_sb, eyej[:, :])

        def col_load(name, src):
            t = wp.tile([128, NG], F32, name=name)
            for g in range(NG):
                nc.sync.dma_start(t[:, g:g + 1], src[g:g + 1, :].rearrange("a b -> b a"))
            return t

        convb_sb = col_load("convb_sb", convb)
        silub_sb = col_load("silub_sb", silub)
        dtb_sb = col_load("dtb_sb", dtb)

        ln2w_sb = wp.tile([128, DM], F32)
        nc.sync.dma_start(ln2w_sb, ln2w[0:1, :].to_broadcast([128, DM]))
        ln2b_sb = wp.tile([128, DM], F32)
        nc.sync.dma_start(ln2b_sb, ln2b[0:1, :].to_broadcast([128, DM]))

        eps_sb = wp.tile([128, 1], F32)
        nc.vector.memset(eps_sb, EPS)
        # per-(g,n) scan carry, column n of block g; zero before chunk 0
        hlast = wp.tile([128, NG, DS], BF16)
        nc.vector.memset(hlast, 0.0)

        ypart = dramp.tile([L, DM], BF16, bufs=1)
        gbuf = dramp.tile([NCH, 2, TC, DM], BF16, bufs=1)

        # conv staging with 3-column causal halo; persistent across chunks so
        # the halo copy reads the previous chunk's columns from the same tile
        xz = [chp.tile([128, TC + 3], BF16, name=f"xz{g}") for g in range(NG)]

        # ---------------- per-chunk pipeline ----------------
        # Sections A (LayerNorm 1) and B (xc-half in_proj into conv staging)
        # for chunk cc; emitted one chunk AHEAD of the scan section so the
        # next chunk's front end overlaps the current chunk's scans.
        def emit_A_B(cc):
            t0 = cc * TC
            xn_t = chp.tile([128, DM // 128, TC], BF16, tag="xn_t")
            if cc == 0:
                xt4 = xt4_pre
            else:
                xt4 = chp.tile([128, TC // 128, DM], BF16, tag="xt4", bufs=1)
                for i in range(TC // 128):
                    nc.sync.dma_start(xt4[:, i, :],
                                      x_d[t0 + i * 128: t0 + (i + 1) * 128, :])
            mv4 = lnp.tile([128, TC // 128, 2], F32, tag="mv4")
            for i in range(TC // 128):
                st = lnp.tile([128, 6], F32, tag="st")
                nc.vector.bn_stats(st, xt4[:, i, :])
                nc.vector.bn_aggr(mv4[:, i, :], st)
            for i in range(TC // 128):
                nc.scalar.activation(mv4[:, i, 1:2], mv4[:, i, 1:2], AF.Ln,
                                     bias=eps_sb[:, 0:1])
            for i in range(TC // 128):
                nc.scalar.activation(mv4[:, i, 1:2], mv4[:, i, 1:2], AF.Exp,
                                     scale=-0.5)
            for i in range(TC // 128):
                nc.vector.tensor_scalar(out=xt4[:, i, :], in0=xt4[:, i, :],
                                        scalar1=mv4[:, i, 0:1],
                                        scalar2=mv4[:, i, 1:2],
                                        op0=OP.subtract, op1=OP.mult)
                ps_t = pp.tile([128, DM], BF16, tag="trps", bufs=1)
                for kb in range(DM // 128):
                    nc.tensor.transpose(ps_t[:, kb * 128:(kb + 1) * 128],
                                        xt4[:, i, kb * 128:(kb + 1) * 128],
                                        eye_sb)
                nc.vector.tensor_copy(
                    xn_t[:, :, i * 128:(i + 1) * 128],
                    ps_t.rearrange("p (k t) -> p k t", k=DM // 128))
            for g in range(NG):
                if cc == 0:
                    nc.vector.memset(xz[g][:, 0:3], 0.0)
                else:
                    nc.vector.tensor_copy(xz[g][:, 0:3], xz[g][:, TC:TC + 3])
                pz = pp.tile([128, TC], F32, tag="ps")
                for k in range(DM // 128):
                    nc.tensor.matmul(pz, w_in_sb[:, k, g * 128:(g + 1) * 128],
                                     xn_t[:, k, :],
                                     start=(k == 0), stop=(k == DM // 128 - 1))
                nc.scalar.copy(xz[g][:, 3:TC + 3], pz)
            return xn_t

        xn_pend = emit_A_B(0)
        for c in range(NCH):
            t0 = c * TC
            xn_t = xn_pend

            # -- B2: in_proj (z half) + SiLU gate --
            sg = [chp.tile([128, TC], BF16, tag=f"sg{g}", name=f"sg{g}") for g in range(NG)]
            for g in range(NG):
                pz = pp.tile([128, TC], F32, tag="ps")
                for k in range(DM // 128):
                    nc.tensor.matmul(
                        pz, w_in_sb[:, k, DI + g * 128: DI + (g + 1) * 128],
                        xn_t[:, k, :],
                        start=(k == 0), stop=(k == DM // 128 - 1))
                _silu(nc, snp, sg[g], pz, silub_sb[:, g:g + 1])

            # -- C: depthwise causal conv + SiLU --
            xc = [chp.tile([128, TC], BF16, tag=f"xc{g}", name=f"xc{g}") for g in range(NG)]
            for g in range(NG):
                pc = pp.tile([128, TC], F32, tag="ps")
                for k in range(D_CONV):
                    nc.tensor.matmul(pc, convd_sb[:, k, g, :],
                                     xz[g][:, k:k + TC],
                                     start=(k == 0), stop=(k == D_CONV - 1))
                _silu(nc, snp, xc[g], pc, convb_sb[:, g:g + 1])

            # -- D: x_proj -> (dt_r, B, C) --
            pxd = pp.tile([64, TC], F32, tag="pxd", bufs=1)
            for g in range(NG):
                nc.tensor.matmul(pxd, w_xp_sb[:, g, :], xc[g],
                                 start=(g == 0), stop=(g == NG - 1))
            xdbl = chp.tile([64, TC], F32R, tag="xdbl", bufs=2)
            nc.scalar.copy(xdbl, pxd)
            bc_d = dramp.tile([2 * DS, TC], BF16, tag="bc_d", bufs=2)
            bc_bf = chp.tile([2 * DS, TC], BF16, tag="bc_bf", bufs=2)
            nc.scalar.copy(bc_bf, xdbl[DR:DR + 2 * DS, :])
            nc.sync.dma_start(bc_d, bc_bf)
            # hoisted per-chunk broadcast of all B (rows 0:DS) and C rows
            # (DS:2*DS) across the 128 partitions
            bcrep = chp.tile([128, 2 * DS, TC], BF16, tag="bcrep", bufs=1)
            nc.sync.dma_start(
                bcrep,
                bc_d.rearrange("n t -> (n t)")[None, :].to_broadcast(
                    [128, 2 * DS * TC]).rearrange("p (n t) -> p n t", n=2 * DS))

            # -- D2: dt = softplus(dt_proj @ dt_r + bias); P = dt * xc --
            # batched by ACT table-set: all Exp first, then all Ln, so the
            # activation table is not reloaded per group.
            dt_c = [chp.tile([128, TC], BF16, tag=f"dt{g}", name=f"dtc{g}") for g in range(NG)]
            p_c = [chp.tile([128, TC], BF16, tag=f"p{g}", name=f"pc{g}") for g in range(NG)]
            for g in range(NG):
                pdt = pp.tile([128, TC], F32, tag="ps")
                nc.tensor.matmul(pdt, w_dt_sb[:, g * 128:(g + 1) * 128],
                                 xdbl[0:DR, :], start=True, stop=True)
                nc.scalar.activation(dt_c[g], pdt, AF.Exp,
                                     bias=dtb_sb[:, g:g + 1])
            for g in range(NG):
                nc.scalar.activation(dt_c[g], dt_c[g], AF.Ln, bias=1.0)
            for g in range(NG):
                nc.vector.tensor_mul(p_c[g], dt_c[g], xc[g])

            # front end of the NEXT chunk, emitted ahead of this chunk's
            # scans so LN1/in_proj overlap the scan phase on idle engines
            if c + 1 < NCH:
                xn_pend = emit_A_B(c + 1)

            # -- E/F: selective scan, one channel block at a time --
            yf = [chp.tile([128, TC], BF16, tag=f"yf{g}", name=f"yf{g}") for g in range(NG)]
            prev_gate = None
            for g in range(NG):
                bt_all = chp.tile([128, DS, TC], BF16, tag="bt_all",
                                  name=f"bt{g}", bufs=1)
                nc.vector.tensor_mul(
                    bt_all,
                    p_c[g][:, None, :].to_broadcast([128, DS, TC]),
                    bcrep[:, 0:DS, :])
                h_all = chp.tile([128, DS, TC], BF16, tag="h_all",
                                 name=f"h{g}", bufs=1)
                py = ppy.tile([128, TC], F32, tag="py", bufs=2)
                # seed with the D-term via a diagonal matmul: py = diag(D) @ xc
                nc.tensor.matmul(py, d_diag_sb[:, g, :], xc[g],
                                 start=True, stop=False)
                for n in range(DS):
                    da = snp.tile([128, TC], F32, tag="da")
                    nc.scalar.activation(da, dt_c[g], AF.Exp,
                                         scale=a_sb[:, g, n:n + 1])
                    nc.vector.tensor_tensor_scan(
                        h_all[:, n, :], da, bt_all[:, n, :],
                        initial=hlast[:, g, n:n + 1],
                        op0=OP.mult, op1=OP.add)
                # save the per-state carry with an on-queue DVE copy: a DMA
                # here costs ~5us of cross-engine semaphore latency at every
                # block boundary (the next block's scans WAR on h_all)
                nc.vector.tensor_copy(hlast[:, g, :], h_all[:, :, TC - 1])
                yn_all = chp.tile([128, DS, TC], BF16, tag="yn_all",
                                  name=f"yn{g}", bufs=1)
                nc.vector.tensor_mul(yn_all, h_all, bcrep[:, DS:2 * DS, :])
                for n in range(DS):
                    nc.tensor.matmul(py, eyen_sb, yn_all[:, n, :],
                                     start=False, stop=(n == DS - 1))
                # gate the PREVIOUS block now: its y-acc matmuls ran while this
                # block scanned, so the DVE never stalls on the tensor engine
                if prev_gate is not None:
                    pg, ppy_t = prev_gate
                    nc.vector.tensor_mul(yf[pg], ppy_t, sg[pg])
                prev_gate = (g, py)
            pg, ppy_t = prev_gate
            nc.vector.tensor_mul(yf[pg], ppy_t, sg[pg])

            # -- H: out_proj, emitted directly in [t, dm] layout --
            for tb in range(TC // 128):
                po = pp.tile([128, DM], F32, tag="ps")
                for g in range(NG):
                    nc.tensor.matmul(po, yf[g][:, tb * 128:(tb + 1) * 128],
                                     w_out_sb[:, g, :],
                                     start=(g == 0), stop=(g == NG - 1))
                ot = lnp.tile([128, DM], BF16, tag="ot2")
                nc.scalar.copy(ot, po)
                nc.sync.dma_start(
                    ypart[t0 + tb * 128: t0 + (tb + 1) * 128, :], ot)
            # per-chunk AllGather of this chunk's partial output; overlaps
            # with the next chunk's compute, so only the last one is a tail
            nc.gpsimd.collective_compute(
                "AllGather", OP.bypass, replica_groups=REPLICA_GROUPS,
                ins=[ypart[t0:t0 + TC, :].opt()], outs=[gbuf[c].opt()],
            )

        # ---------------- combine directions + final LayerNorm ----------------
        # Natural chunk cn reads the dir-0 partial from gbuf[cn, 0] and the
        # dir-1 (flipped) partial from gbuf[NCH-1-cn, 1]. Chunks 1 and 2 need
        # only the earlier collectives, so process them first; only chunks
        # {3, 0} wait on the last collective.
        for cn in (1, 2, 3, 0):
            for it in range(TC // 128):
                i = cn * (TC // 128) + it
                c1 = NCH - 1 - cn
                o1 = TC - (it + 1) * 128
                s0 = lnp.tile([128, DM], BF16, tag="xt")
                nc.sync.dma_start(s0, gbuf[cn, 0, it * 128:(it + 1) * 128, :])
                # dir-1 partial is in flipped time order: load the mirrored
                # block forward, then reverse rows via the exchange matrix on
                # the tensor engine (DMA cannot step partitions backwards)
                s1 = lnp.tile([128, DM], BF16, tag="ot")
                nc.sync.dma_start(s1, gbuf[c1, 1, o1:o1 + 128, :])
                pj = pp.tile([128, DM], F32, tag="ps")
                nc.tensor.matmul(pj, eyej_sb, s1, start=True, stop=True)
                xt2 = lnp.tile([128, DM], F32, tag="xt2")
                nc.sync.dma_start(xt2, x_nat[i * 128:(i + 1) * 128, :])
                sf = lnp.tile([128, DM], F32, tag="sf")
                nc.vector.tensor_add(sf, pj, s0)
                nc.vector.tensor_add(sf, sf, xt2)
                st = lnp.tile([128, 6], F32, tag="st")
                nc.vector.bn_stats(st, sf)
                mv = lnp.tile([128, 2], F32, tag="mv")
                nc.vector.bn_aggr(mv, st)
                nc.scalar.activation(mv[:, 1:2], mv[:, 1:2], AF.Ln,
                                     bias=eps_sb[:, 0:1])
                nc.scalar.activation(mv[:, 1:2], mv[:, 1:2], AF.Exp, scale=-0.5)
                nc.vector.tensor_scalar(out=sf, in0=sf, scalar1=mv[:, 0:1],
                                        scalar2=mv[:, 1:2],
                                        op0=OP.subtract, op1=OP.mult)
                nc.vector.tensor_mul(sf, sf, ln2w_sb)
                nc.vector.tensor_add(sf, sf, ln2b_sb)
                nc.sync.dma_start(out[i * 128:(i + 1) * 128, :], sf)


# ---------------- host side ----------------

def make_core_inputs(inputs):
    """Build the 8 per-core input dicts from the full problem inputs."""
    x = np.ascontiguousarray(inputs["x"], dtype=np.float32)        # [B, L, DM]
    ln1_w = inputs["ln1_w"].astype(np.float32)
    ln1_b = inputs["ln1_b"].astype(np.float32)
    in_w = inputs["in_proj_w"].astype(np.float32)                  # [2, 2DI, DM]
    conv_w = inputs["conv_w"].astype(np.float32)                   # [2, DI, 4]
    conv_b = inputs["conv_b"].astype(np.float32)                   # [2, DI]
    xp_w = inputs["x_proj_w"].astype(np.float32)                   # [2, 64, DI]
    dt_w = inputs["dt_proj_w"].astype(np.float32)                  # [2, DI, DR]
    dt_b = inputs["dt_proj_b"].astype(np.float32)                  # [2, DI]
    a_log = inputs["A_log"].astype(np.float32)                     # [2, DI, DS]
    d_par = inputs["D_param"].astype(np.float32)                   # [2, DI]
    out_w = inputs["out_proj_w"].astype(np.float32)                # [2, DM, DI]
    ln2_w = inputs["ln2_w"].astype(np.float32)
    ln2_b = inputs["ln2_b"].astype(np.float32)

    bf16 = ml_dtypes.bfloat16
    eye = np.eye(128, dtype=np.float32)
    per_dir = []
    for d in range(2):
        w = in_w[d]                                   # [2DI, DM]
        w_in_T = np.ascontiguousarray((w * ln1_w[None, :]).T)      # [DM, 2DI]
        v = w @ ln1_b                                  # [2DI]
        csum = conv_w[d].sum(axis=1)                   # [DI]
        convb_adj = conv_b[d] + csum * v[:DI]
        silub_adj = v[DI:]
        convd = np.zeros((D_CONV, NG, 128, 128), np.float32)
        for k in range(D_CONV):
            for g in range(NG):
                np.fill_diagonal(convd[k, g], conv_w[d, g * 128:(g + 1) * 128, k])
        d_diag = np.zeros((NG, 128, 128), np.float32)
        for g in range(NG):
            np.fill_diagonal(d_diag[g], d_par[d, g * 128:(g + 1) * 128])
        a_neg = -np.exp(a_log[d])                      # [DI, DS]
        per_dir.append(dict(
            w_in=w_in_T.astype(bf16),
            convd=convd.astype(bf16),
            convb=convb_adj.reshape(NG, 128),
            silub=silub_adj.reshape(NG, 128),
            w_xp=np.ascontiguousarray(xp_w[d].T).astype(bf16),     # [DI, 64]
            w_dt=np.ascontiguousarray(dt_w[d].T),      # [DR, DI]
            dtb=dt_b[d].reshape(NG, 128),
            a_sc=np.ascontiguousarray(a_neg.reshape(NG, 128, DS)),
            d_diag=d_diag.astype(bf16),
            w_out=np.ascontiguousarray(out_w[d].T).astype(bf16),   # [DI, DM]
        ))

    in_maps = []
    for core in range(N_CORES):
        d, b = core // 4, core % 4
        xb = x[b]
        m = dict(per_dir[d])
        m["x_d"] = (xb if d == 0 else np.ascontiguousarray(xb[::-1])).astype(bf16)
        m["x_nat"] = xb
        m["eye"] = eye.astype(bf16)
        m["eyen"] = eye.astype(bf16)
        m["eyej"] = eye[::-1].astype(bf16)
        m["ln2w"] = ln2_w.reshape(1, DM)
        m["ln2b"] = ln2_b.reshape(1, DM)
        in_maps.append(m)
    return in_maps


_NC = None


def _get_module():
    global _NC
    if _NC is None:
        _NC = build_module()
    return _NC


def kernel(**inputs) -> np.ndarray:
    nc = _get_module()
    in_maps = make_core_inputs(inputs)
    res = run_bass_kernel_spmd(nc, in_maps, core_ids=list(range(N_CORES)))
    outs = [res.results[b]["out"] for b in range(B_SZ)]
    return np.stack(outs, axis=0)


if __name__ == "__main__":
    nc = build_module()
    print("module built ok")


# revision 21
# speedup vs baseline: 1.0227x; 1.0034x over previous
"""Bidirectional Mamba layer on 8 Trainium2 NeuronCores (Bass/Tile).

Sharding: 8 cores = 2 directions x 4 batch samples. Each core runs the full
Mamba block for its (direction, sample) pair; a pairwise AllGather combines
the two directions, and every core (redundantly, SPMD-symmetric) applies the
final residual + LayerNorm.

Layout convention on device: channels on partitions, time on the free axis.
The selective scan runs as DVE tensor_tensor_scan (one recurrence per
(channel, state) partition row, time along free). Per channel-block g the
16 state recurrences share batched B/C multiplies: bt_all/yn_all are single
[128, DS*TC] DVE passes using a stride-0 free broadcast of p / direct reads
of a per-chunk broadcast tile of B and C rows.
"""

import ml_dtypes
import numpy as np

import concourse.bass as bass
import concourse.bacc as bacc
import concourse.tile as tile
from concourse import mybir
from concourse.bass_utils import run_bass_kernel_spmd

# ---- problem shapes (hardcoded per contract) ----
B_SZ, L, DM = 4, 2048, 512
D_CONV, DS, DR = 4, 16, 32
DI = 1024                  # d_inner
E2 = 2 * DI                # in_proj rows
NG = DI // 128             # 8 channel blocks
TC = 512                   # time chunk
NCH = L // TC              # 4 chunks
NTT = L // 128             # 16 time tiles of 128
EPS = 1e-5
F32 = mybir.dt.float32
F32R = mybir.dt.float32r
BF16 = mybir.dt.bfloat16
AF = mybir.ActivationFunctionType
OP = mybir.AluOpType

N_CORES = 8

# CoreSim does not implement Silu; tests flip this to use sigmoid+mult
USE_SILU = True
REPLICA_GROUPS = [[0, 4], [1, 5], [2, 6], [3, 7]]


def _silu(nc, pool, out_tile, psum, bias):
    """out = silu(psum + bias); Silu on HW, sigmoid+mult fallback for CoreSim."""
    if USE_SILU:
        nc.scalar.activation(out_tile, psum, AF.Silu, bias=bias)
    else:
        zb = pool.tile(list(out_tile.shape), F32, tag="_silu_zb", name="zb",
                       bufs=1)
        nc.scalar.activation(zb, psum, AF.Identity, bias=bias)
        nc.scalar.activation(out_tile, psum, AF.Sigmoid, bias=bias)
        nc.vector.tensor_mul(out_tile, zb, out_tile)


def build_module():
    nc = bacc.Bacc(
        "TRN2", target_bir_lowering=False, debug=False, num_devices=N_CORES
    )

    # ---------------- I/O ----------------
    x_d = nc.dram_tensor("x_d", [L, DM], BF16, kind="ExternalInput")
    x_nat = nc.dram_tensor("x_nat", [L, DM], F32, kind="ExternalInput")
    w_in = nc.dram_tensor("w_in", [DM, E2], BF16, kind="ExternalInput")
    convd = nc.dram_tensor("convd", [D_CONV, NG, 128, 128], BF16, kind="ExternalInput")
    convb = nc.dram_tensor("convb", [NG, 128], F32, kind="ExternalInput")
    silub = nc.dram_tensor("silub", [NG, 128], F32, kind="ExternalInput")
    w_xp = nc.dram_tensor("w_xp", [DI, DR + 2 * DS], BF16, kind="ExternalInput")
    w_dt = nc.dram_tensor("w_dt", [DR, DI], F32R, kind="ExternalInput")
    dtb = nc.dram_tensor("dtb", [NG, 128], F32, kind="ExternalInput")
    a_sc = nc.dram_tensor("a_sc", [NG, 128, DS], F32, kind="ExternalInput")
    d_diag = nc.dram_tensor("d_diag", [NG, 128, 128], BF16, kind="ExternalInput")
    w_out = nc.dram_tensor("w_out", [DI, DM], BF16, kind="ExternalInput")
    eye = nc.dram_tensor("eye", [128, 128], BF16, kind="ExternalInput")
    eyen = nc.dram_tensor("eyen", [128, 128], BF16, kind="ExternalInput")
    eyej = nc.dram_tensor("eyej", [128, 128], BF16, kind="ExternalInput")
    ln2w = nc.dram_tensor("ln2w", [1, DM], F32, kind="ExternalInput")
    ln2b = nc.dram_tensor("ln2b", [1, DM], F32, kind="ExternalInput")
    out = nc.dram_tensor("out", [L, DM], F32, kind="ExternalOutput")

    with tile.TileContext(nc) as tc:
        build_program(
            tc, x_d, x_nat, w_in, convd, convb, silub, w_xp, w_dt, dtb,
            a_sc, d_diag, w_out, eye, eyen, eyej, ln2w, ln2b, out,
        )
    nc.compile()
    return nc


def build_program(tc, x_d, x_nat, w_in, convd, convb, silub, w_xp, w_dt, dtb,
                  a_sc, d_diag, w_out, eye, eyen, eyej, ln2w, ln2b, out):
    nc = tc.nc
    from contextlib import ExitStack

    ctx = ExitStack()
    with ctx:
        wp = ctx.enter_context(tc.tile_pool(name="wp", bufs=1))
        lnp = ctx.enter_context(tc.tile_pool(name="lnp", bufs=2))
        chp = ctx.enter_context(tc.tile_pool(name="chp", bufs=1))
        snp = ctx.enter_context(tc.tile_pool(name="snp", bufs=2))
        pp = ctx.enter_context(tc.tile_pool(name="pp", space="PSUM", bufs=3))
        ppy = ctx.enter_context(tc.tile_pool(name="ppy", space="PSUM", bufs=3))
        dramp = ctx.enter_context(tc.tile_pool(name="dramp", bufs=2, space="DRAM"))

        # ---------------- prefetch chunk-0 input ahead of the weights ----
        xt4_pre = chp.tile([128, TC // 128, DM], BF16, tag="xt4", bufs=1)
        for i in range(TC // 128):
            nc.sync.dma_start(xt4_pre[:, i, :],
                              x_d[i * 128:(i + 1) * 128, :])

        # ---------------- load weights ----------------
        w_in_sb = wp.tile([128, DM // 128, E2], BF16)
        nc.sync.dma_start(w_in_sb, w_in.rearrange("(k p) e -> p k e", p=128))
        convd_sb = wp.tile([128, D_CONV, NG, 128], BF16)
        for k in range(D_CONV):
            nc.sync.dma_start(convd_sb[:, k, :, :], convd[k].rearrange("g p e -> p g e"))
        w_xp_sb = wp.tile([128, NG, DR + 2 * DS], BF16)
        nc.sync.dma_start(w_xp_sb, w_xp.rearrange("(g p) e -> p g e", p=128))
        w_dt_sb = wp.tile([DR, DI], F32R)
        nc.sync.dma_start(w_dt_sb, w_dt[:, :])
        w_out_sb = wp.tile([128, NG, DM], BF16)
        nc.sync.dma_start(w_out_sb, w_out.rearrange("(g p) e -> p g e", p=128))
        a_sb = wp.tile([128, NG, DS], F32)
        nc.sync.dma_start(a_sb, a_sc.rearrange("g p e -> p g e"))
        d_diag_sb = wp.tile([128, NG, 128], BF16)
        nc.sync.dma_start(d_diag_sb, d_diag.rearrange("g p e -> p g e"))
        eye_sb = wp.tile([128, 128], BF16)
        nc.sync.dma_start(eye_sb, eye[:, :])
        eyen_sb = wp.tile([128, 128], BF16)
        nc.sync.dma_start(eyen_sb, eyen[:, :])
        eyej_sb = wp.tile([128, 128], BF16)
        nc.sync.dma_start(eyej_sb, eyej[:, :])

        def col_load(name, src):
            t = wp.tile([128, NG], F32, name=name)
            for g in range(NG):
                nc.sync.dma_start(t[:, g:g + 1], src[g:g + 1, :].rearrange("a b -> b a"))
            return t

        convb_sb = col_load("convb_sb", convb)
        silub_sb = col_load("silub_sb", silub)
        dtb_sb = col_load("dtb_sb", dtb)

        ln2w_sb = wp.tile([128, DM], F32)
        nc.sync.dma_start(ln2w_sb, ln2w[0:1, :].to_broadcast([128, DM]))
        ln2b_sb = wp.tile([128, DM], F32)
        nc.sync.dma_start(ln2b_sb, ln2b[0:1, :].to_broadcast([128, DM]))

        eps_sb = wp.tile([128, 1], F32)
        nc.vector.memset(eps_sb, EPS)
        # per-(g,n) scan carry, column n of block g; zero before chunk 0
        hlast = wp.tile([128, NG, DS], BF16)
        nc.vector.memset(hlast, 0.0)

        ypart = dramp.tile([L, DM], BF16, bufs=1)
        gbuf = dramp.tile([NCH, 2, TC, DM], BF16, bufs=1)

        # conv staging with 3-column causal halo; persistent across chunks so
        # the halo copy reads the previous chunk's columns from the same tile
        xz = [chp.tile([128, TC + 3], BF16, name=f"xz{g}") for g in range(NG)]

        # ---------------- per-chunk pipeline ----------------
        # Sections A (LayerNorm 1) and B (xc-half in_proj into conv staging)
        # for chunk cc; emitted one chunk AHEAD of the scan section so the
        # next chunk's front end overlaps the current chunk's scans.
        def emit_A_B(cc):
            t0 = cc * TC
            xn_t = chp.tile([128, DM // 128, TC], BF16, tag="xn_t")
            if cc == 0:
                xt4 = xt4_pre
            else:
                xt4 = chp.tile([128, TC // 128, DM], BF16, tag="xt4", bufs=1)
                for i in range(TC // 128):
                    nc.sync.dma_start(xt4[:, i, :],
                                      x_d[t0 + i * 128: t0 + (i + 1) * 128, :])
            mv4 = lnp.tile([128, TC // 128, 2], F32, tag="mv4")
            for i in range(TC // 128):
                st = lnp.tile([128, 6], F32, tag="st")
                nc.vector.bn_stats(st, xt4[:, i, :])
                nc.vector.bn_aggr(mv4[:, i, :], st)
            for i in range(TC // 128):
                nc.scalar.activation(mv4[:, i, 1:2], mv4[:, i, 1:2], AF.Ln,
                                     bias=eps_sb[:, 0:1])
            for i in range(TC // 128):
                nc.scalar.activation(mv4[:, i, 1:2], mv4[:, i, 1:2], AF.Exp,
                                     scale=-0.5)
            for i in range(TC // 128):
                nc.vector.tensor_scalar(out=xt4[:, i, :], in0=xt4[:, i, :],
                                        scalar1=mv4[:, i, 0:1],
                                        scalar2=mv4[:, i, 1:2],
                                        op0=OP.subtract, op1=OP.mult)
                ps_t = pp.tile([128, DM], BF16, tag="trps", bufs=1)
                for kb in range(DM // 128):
                    nc.tensor.transpose(ps_t[:, kb * 128:(kb + 1) * 128],
                                        xt4[:, i, kb * 128:(kb + 1) * 128],
                                        eye_sb)
                nc.vector.tensor_copy(
                    xn_t[:, :, i * 128:(i + 1) * 128],
                    ps_t.rearrange("p (k t) -> p k t", k=DM // 128))
            for g in range(NG):
                if cc == 0:
                    nc.vector.memset(xz[g][:, 0:3], 0.0)
                else:
                    nc.vector.tensor_copy(xz[g][:, 0:3], xz[g][:, TC:TC + 3])
                pz = pp.tile([128, TC], F32, tag="ps")
                for k in range(DM // 128):
                    nc.tensor.matmul(pz, w_in_sb[:, k, g * 128:(g + 1) * 128],
                                     xn_t[:, k, :],
                                     start=(k == 0), stop=(k == DM // 128 - 1))
                nc.scalar.copy(xz[g][:, 3:TC + 3], pz)
            return xn_t

        xn_pend = emit_A_B(0)
        for c in range(NCH):
            t0 = c * TC
            xn_t = xn_pend

            # -- B2: in_proj (z half) + SiLU gate --
            sg = [chp.tile([128, TC], BF16, tag=f"sg{g}", name=f"sg{g}") for g in range(NG)]
            for g in range(NG):
                pz = pp.tile([128, TC], F32, tag="ps")
                for k in range(DM // 128):
                    nc.tensor.matmul(
                        pz, w_in_sb[:, k, DI + g * 128: DI + (g + 1) * 128],
                        xn_t[:, k, :],
                        start=(k == 0), stop=(k == DM // 128 - 1))
                _silu(nc, snp, sg[g], pz, silub_sb[:, g:g + 1])

            # -- C: depthwise causal conv + SiLU --
            xc = [chp.tile([128, TC], BF16, tag=f"xc{g}", name=f"xc{g}") for g in range(NG)]
            for g in range(NG):
                pc = pp.tile([128, TC], F32, tag="ps")
                for k in range(D_CONV):
                    nc.tensor.matmul(pc, convd_sb[:, k, g, :],
                                     xz[g][:, k:k + TC],
                                     start=(k == 0), stop=(k == D_CONV - 1))
                _silu(nc, snp, xc[g], pc, convb_sb[:, g:g + 1])

            # -- D: x_proj -> (dt_r, B, C) --
            pxd = pp.tile([64, TC], F32, tag="pxd", bufs=1)
            for g in range(NG):
                nc.tensor.matmul(pxd, w_xp_sb[:, g, :], xc[g],
                                 start=(g == 0), stop=(g == NG - 1))
            xdbl = chp.tile([64, TC], F32R, tag="xdbl", bufs=2)
            nc.scalar.copy(xdbl, pxd)
            bc_d = dramp.tile([2 * DS, TC], BF16, tag="bc_d", bufs=2)
            bc_bf = chp.tile([2 * DS, TC], BF16, tag="bc_bf", bufs=2)
            nc.scalar.copy(bc_bf, xdbl[DR:DR + 2 * DS, :])
            nc.sync.dma_start(bc_d, bc_bf)
            # hoisted per-chunk broadcast of all B (rows 0:DS) and C rows
            # (DS:2*DS) across the 128 partitions
            bcrep = chp.tile([128, 2 * DS, TC], BF16, tag="bcrep", bufs=1)
            nc.sync.dma_start(
                bcrep,
                bc_d.rearrange("n t -> (n t)")[None, :].to_broadcast(
                    [128, 2 * DS * TC]).rearrange("p (n t) -> p n t", n=2 * DS))

            # -- D2: dt = softplus(dt_proj @ dt_r + bias); P = dt * xc --
            # batched by ACT table-set: all Exp first, then all Ln, so the
            # activation table is not reloaded per group.
            dt_c = [chp.tile([128, TC], BF16, tag=f"dt{g}", name=f"dtc{g}") for g in range(NG)]
            p_c = [chp.tile([128, TC], BF16, tag=f"p{g}", name=f"pc{g}") for g in range(NG)]
            for g in range(NG):
                pdt = pp.tile([128, TC], F32, tag="ps")
                nc.tensor.matmul(pdt, w_dt_sb[:, g * 128:(g + 1) * 128],
                                 xdbl[0:DR, :], start=True, stop=True)
                nc.scalar.activation(dt_c[g], pdt, AF.Exp,
                                     bias=dtb_sb[:, g:g + 1])
            for g in range(NG):
                nc.scalar.activation(dt_c[g], dt_c[g], AF.Ln, bias=1.0)
            for g in range(NG):
                nc.vector.tensor_mul(p_c[g], dt_c[g], xc[g])

            # front end of the NEXT chunk, emitted ahead of this chunk's
            # scans so LN1/in_proj overlap the scan phase on idle engines
            if c + 1 < NCH:
                xn_pend = emit_A_B(c + 1)

            # -- E/F: selective scan, one channel block at a time --
            yf = [chp.tile([128, TC], BF16, tag=f"yf{g}", name=f"yf{g}") for g in range(NG)]
            prev_gate = None
            for g in range(NG):
                bt_all = chp.tile([128, DS, TC], BF16, tag="bt_all",
                                  name=f"bt{g}", bufs=1)
                nc.vector.tensor_mul(
                    bt_all,
                    p_c[g][:, None, :].to_broadcast([128, DS, TC]),
                    bcrep[:, 0:DS, :])
                h_all = chp.tile([128, DS, TC], BF16, tag="h_all",
                                 name=f"h{g}", bufs=1)
                py = ppy.tile([128, TC], F32, tag="py", bufs=2)
                # seed with the D-term via a diagonal matmul: py = diag(D) @ xc
                nc.tensor.matmul(py, d_diag_sb[:, g, :], xc[g],
                                 start=True, stop=False)
                for n in range(DS):
                    da = snp.tile([128, TC], F32, tag="da")
                    nc.scalar.activation(da, dt_c[g], AF.Exp,
                                         scale=a_sb[:, g, n:n + 1])
                    nc.vector.tensor_tensor_scan(
                        h_all[:, n, :], da, bt_all[:, n, :],
                        initial=hlast[:, g, n:n + 1],
                        op0=OP.mult, op1=OP.add)
                # save the per-state carry with an on-queue DVE copy: a DMA
                # here costs ~5us of cross-engine semaphore latency at every
                # block boundary (the next block's scans WAR on h_all)
                nc.vector.tensor_copy(hlast[:, g, :], h_all[:, :, TC - 1])
                yn_all = chp.tile([128, DS, TC], BF16, tag="yn_all",
                                  name=f"yn{g}", bufs=1)
                nc.vector.tensor_mul(yn_all, h_all, bcrep[:, DS:2 * DS, :])
                for n in range(DS):
                    nc.tensor.matmul(py, eyen_sb, yn_all[:, n, :],
                                     start=False, stop=(n == DS - 1))
                # gate the PREVIOUS block now: its y-acc matmuls ran while this
                # block scanned, so the DVE never stalls on the tensor engine
                if prev_gate is not None:
                    pg, ppy_t = prev_gate
                    nc.vector.tensor_mul(yf[pg], ppy_t, sg[pg])
                prev_gate = (g, py)
            pg, ppy_t = prev_gate
            nc.vector.tensor_mul(yf[pg], ppy_t, sg[pg])

            # -- H: out_proj, emitted directly in [t, dm] layout --
            for tb in range(TC // 128):
                po = pp.tile([128, DM], F32, tag="ps")
                for g in range(NG):
                    nc.tensor.matmul(po, yf[g][:, tb * 128:(tb + 1) * 128],
                                     w_out_sb[:, g, :],
                                     start=(g == 0), stop=(g == NG - 1))
                ot = lnp.tile([128, DM], BF16, tag="ot2")
                nc.scalar.copy(ot, po)
                nc.sync.dma_start(
                    ypart[t0 + tb * 128: t0 + (tb + 1) * 128, :], ot)
            # per-chunk AllGather of this chunk's partial output; overlaps
            # with the next chunk's compute, so only the last one is a tail
            nc.gpsimd.collective_compute(
                "AllGather", OP.bypass, replica_groups=REPLICA_GROUPS,
                ins=[ypart[t0:t0 + TC, :].opt()], outs=[gbuf[c].opt()],
            )

        # ---------------- combine directions + final LayerNorm ----------------
        # Natural chunk cn reads the dir-0 partial from gbuf[cn, 0] and the
        # dir-1 (flipped) partial from gbuf[NCH-1-cn, 1]. Chunks 1 and 2 need
        # only the earlier collectives, so process them first; only chunks
        # {3, 0} wait on the last collective.
        for cn in (1, 2, 3, 0):
            for it in range(TC // 128):
                i = cn * (TC // 128) + it
                c1 = NCH - 1 - cn
                o1 = TC - (it + 1) * 128
                s0 = lnp.tile([128, DM], BF16, tag="xt")
                nc.sync.dma_start(s0, gbuf[cn, 0, it * 128:(it + 1) * 128, :])
                # dir-1 partial is in flipped time order: load the mirrored
                # block forward, then reverse rows via the exchange matrix on
                # the tensor engine (DMA cannot step partitions backwards)
                s1 = lnp.tile([128, DM], BF16, tag="ot")
                nc.sync.dma_start(s1, gbuf[c1, 1, o1:o1 + 128, :])
                pj = pp.tile([128, DM], F32, tag="ps")
                nc.tensor.matmul(pj, eyej_sb, s1, start=True, stop=True)
                xt2 = lnp.tile([128, DM], F32, tag="xt2")
                nc.sync.dma_start(xt2, x_nat[i * 128:(i + 1) * 128, :])
                sf = lnp.tile([128, DM], F32, tag="sf")
                nc.vector.tensor_add(sf, pj, s0)
                nc.vector.tensor_add(sf, sf, xt2)
                st = lnp.tile([128, 6], F32, tag="st")
                nc.vector.bn_stats(st, sf)
                mv = lnp.tile([128, 2], F32, tag="mv")
                nc.vector.bn_aggr(mv, st)
                nc.scalar.activation(mv[:, 1:2], mv[:, 1:2], AF.Ln,
                                     bias=eps_sb[:, 0:1])
                nc.scalar.activation(mv[:, 1:2], mv[:, 1:2], AF.Exp, scale=-0.5)
                nc.vector.tensor_scalar(out=sf, in0=sf, scalar1=mv[:, 0:1],
                                        scalar2=mv[:, 1:2],
                                        op0=OP.subtract, op1=OP.mult)
                nc.vector.tensor_mul(sf, sf, ln2w_sb)
                nc.vector.tensor_add(sf, sf, ln2b_sb)
                nc.sync.dma_start(out[i * 128:(i + 1) * 128, :], sf)


# ---------------- host side ----------------

def make_core_inputs(inputs):
    """Build the 8 per-core input dicts from the full problem inputs."""
    x = np.ascontiguousarray(inputs["x"], dtype=np.float32)        # [B, L, DM]
    ln1_w = inputs["ln1_w"].astype(np.float32)
    ln1_b = inputs["ln1_b"].astype(np.float32)
    in_w = inputs["in_proj_w"].astype(np.float32)                  # [2, 2DI, DM]
    conv_w = inputs["conv_w"].astype(np.float32)                   # [2, DI, 4]
    conv_b = inputs["conv_b"].astype(np.float32)                   # [2, DI]
    xp_w = inputs["x_proj_w"].astype(np.float32)                   # [2, 64, DI]
    dt_w = inputs["dt_proj_w"].astype(np.float32)                  # [2, DI, DR]
    dt_b = inputs["dt_proj_b"].astype(np.float32)                  # [2, DI]
    a_log = inputs["A_log"].astype(np.float32)                     # [2, DI, DS]
    d_par = inputs["D_param"].astype(np.float32)                   # [2, DI]
    out_w = inputs["out_proj_w"].astype(np.float32)                # [2, DM, DI]
    ln2_w = inputs["ln2_w"].astype(np.float32)
    ln2_b = inputs["ln2_b"].astype(np.float32)

    bf16 = ml_dtypes.bfloat16
    eye = np.eye(128, dtype=np.float32)
    per_dir = []
    for d in range(2):
        w = in_w[d]                                   # [2DI, DM]
        w_in_T = np.ascontiguousarray((w * ln1_w[None, :]).T)      # [DM, 2DI]
        v = w @ ln1_b                                  # [2DI]
        csum = conv_w[d].sum(axis=1)                   # [DI]
        convb_adj = conv_b[d] + csum * v[:DI]
        silub_adj = v[DI:]
        convd = np.zeros((D_CONV, NG, 128, 128), np.float32)
        for k in range(D_CONV):
            for g in range(NG):
                np.fill_diagonal(convd[k, g], conv_w[d, g * 128:(g + 1) * 128, k])
        d_diag = np.zeros((NG, 128, 128), np.float32)
        for g in range(NG):
            np.fill_diagonal(d_diag[g], d_par[d, g * 128:(g + 1) * 128])
        a_neg = -np.exp(a_log[d])                      # [DI, DS]
        per_dir.append(dict(
            w_in=w_in_T.astype(bf16),
            convd=convd.astype(bf16),
            convb=convb_adj.reshape(NG, 128),
            silub=silub_adj.reshape(NG, 128),
            w_xp=np.ascontiguousarray(xp_w[d].T).astype(bf16),     # [DI, 64]
            w_dt=np.ascontiguousarray(dt_w[d].T),      # [DR, DI]
            dtb=dt_b[d].reshape(NG, 128),
            a_sc=np.ascontiguousarray(a_neg.reshape(NG, 128, DS)),
            d_diag=d_diag.astype(bf16),
            w_out=np.ascontiguousarray(out_w[d].T).astype(bf16),   # [DI, DM]
        ))

    in_maps = []
    for core in range(N_CORES):
        d, b = core // 4, core % 4
        xb = x[b]
        m = dict(per_dir[d])
        m["x_d"] = (xb if d == 0 else np.ascontiguousarray(xb[::-1])).astype(bf16)
        m["x_nat"] = xb
        m["eye"] = eye.astype(bf16)
        m["eyen"] = eye.astype(bf16)
        m["eyej"] = eye[::-1].astype(bf16)
        m["ln2w"] = ln2_w.reshape(1, DM)
        m["ln2b"] = ln2_b.reshape(1, DM)
        in_maps.append(m)
    return in_maps


_NC = None


def _get_module():
    global _NC
    if _NC is None:
        _NC = build_module()
    return _NC


def kernel(**inputs) -> np.ndarray:
    nc = _get_module()
    in_maps = make_core_inputs(inputs)
    res = run_bass_kernel_spmd(nc, in_maps, core_ids=list(range(N_CORES)))
    outs = [res.results[b]["out"] for b in range(B_SZ)]
    return np.stack(outs, axis=0)


if __name__ == "__main__":
    nc = build_module()
    print("module built ok")
